# revision 1
# baseline (speedup 1.0000x reference)
"""Multi-head attention (b=2, n=2048, d_model=1024, H=16, d_k=d_v=64) on 8
Trainium2 NeuronCores.

Sharding: 8 cores = 2 (batch) x 4 (head groups of 4 heads).  Each core
computes, for its batch ib and head group g:

    q/k projections in transposed form  qT = Wq_g @ x^T   [256, 2048]
    v projection in natural form        V  = x @ Wv_g^T   [2048, 256]
    per head: S^T = K_h Q_h^T (k on partitions), A^T = exp(S^T/8),
              O^T|Z = [V_h|1]^T A^T  (PE row 64 gives softmax denom Z)
    normalize O^T by 1/Z, out-projection Y^T = Wo_g @ O_cat^T  [1024, 2048]

Host sums the 4 per-group partial Y^T per batch and adds bo.

All matmuls run as float32r (TF32-like, ~1.6e-4 relative error measured on
hw).  Softmax skips the max-subtraction: scores*scale are ~N(0,1) so exp
never overflows in fp32.
"""

import numpy as np
from contextlib import ExitStack

import concourse.bass as bass
import concourse.mybir as mybir
import concourse.tile as tile
from concourse import bacc
from concourse.bass_utils import run_bass_kernel_spmd

F32 = mybir.dt.float32
F32R = mybir.dt.float32r
EXP = mybir.ActivationFunctionType.Exp
ADD = mybir.AluOpType.add
MULT = mybir.AluOpType.mult

D_MODEL = 1024
H = 16
DK = 64
B = 2
N = 2048           # nq = nk
G = 4              # head groups (cores per batch)
HG = H // G        # heads per group = 4
DG = HG * DK       # 256 group dims
KT = 8             # D_MODEL / 128 contraction tiles
NKT = N // 128     # 16 k-tiles in attention
QC = 1024          # attention q-chunk
NCH = N // QC      # 2 chunks
P = 128

_PROGRAM = None


def _build_program():
    nc = bacc.Bacc("TRN2", target_bir_lowering=False, debug=False, num_devices=8)

    # pre-tiled on host: [n-chunk, p, k, 512] so each DMA partition line is
    # one fully contiguous 16 KiB read
    xqT = nc.dram_tensor("xqT", [4, P, KT, 512], F32, kind="ExternalInput").ap()
    xkT = nc.dram_tensor("xkT", [4, P, KT, 512], F32, kind="ExternalInput").ap()
    xvT = nc.dram_tensor("xvT", [NKT, P, KT, P], F32, kind="ExternalInput").ap()
    wqT = nc.dram_tensor("wqT", [P, KT, DG], F32, kind="ExternalInput").ap()
    wkT = nc.dram_tensor("wkT", [P, KT, DG], F32, kind="ExternalInput").ap()
    wvT = nc.dram_tensor("wvT", [P, KT, DG], F32, kind="ExternalInput").ap()
    woT = nc.dram_tensor("woT", [P, 2, D_MODEL], F32, kind="ExternalInput").ap()
    bq_d = nc.dram_tensor("bq_s", [DG], F32, kind="ExternalInput").ap()
    bk_d = nc.dram_tensor("bk_s", [DG], F32, kind="ExternalInput").ap()
    bv_d = nc.dram_tensor("bv_s", [DG], F32, kind="ExternalInput").ap()
    ones_d = nc.dram_tensor("ones_c", [P], F32, kind="ExternalInput").ap()
    yT_d = nc.dram_tensor("yT", [D_MODEL, N], F32, kind="ExternalOutput").ap()
    # dram staging for softmax denominators (internal DRAM tiles fail to load
    # under the axon PJRT path, so an ExternalOutput buffer instead)
    z_st = nc.dram_tensor("z_st", [16, 512], F32, kind="ExternalOutput").ap()

    bq_v = bq_d.rearrange("(j p) -> p j", p=P)        # [128, 2]
    bk_v = bk_d.rearrange("(j p) -> p j", p=P)

    with tile.TileContext(nc) as tc:
        with ExitStack() as ctx:
            const = ctx.enter_context(tc.tile_pool(name="const", bufs=1))
            xin = ctx.enter_context(tc.tile_pool(name="xin", bufs=2))
            xvp = ctx.enter_context(tc.tile_pool(name="xvp", bufs=3))
            work = ctx.enter_context(tc.tile_pool(name="work", bufs=2))
            atp = ctx.enter_context(tc.tile_pool(name="atp", bufs=4))
            smal = ctx.enter_context(tc.tile_pool(name="smal", bufs=4))
            big = ctx.enter_context(tc.tile_pool(name="big", bufs=2, space="PSUM"))
            avp = ctx.enter_context(tc.tile_pool(name="avp", bufs=4, space="PSUM"))

            # ---- constants (weights on the ACT HWDGE queue so they overlap
            # the first x chunks on the SP queue) ----
            wq_sb = const.tile([P, KT, DG], F32R, tag="wq")
            wk_sb = const.tile([P, KT, DG], F32R, tag="wk")
            wv_sb = const.tile([P, KT, DG], F32R, tag="wv")
            wo_sb = const.tile([P, 2, D_MODEL], F32R, tag="wo")
            nc.scalar.dma_start(wk_sb[:], wkT.bitcast(F32R))
            nc.scalar.dma_start(wv_sb[:], wvT.bitcast(F32R))
            nc.scalar.dma_start(wq_sb[:], wqT.bitcast(F32R))
            nc.scalar.dma_start(wo_sb[:], woT.bitcast(F32R))
            bq_sb = const.tile([P, 2], F32, tag="bq")
            bk_sb = const.tile([P, 2], F32, tag="bk")
            nc.scalar.dma_start(bq_sb[:], bq_v)
            nc.scalar.dma_start(bk_sb[:], bk_v)
            bv_sb = const.tile([1, DG], F32R, tag="bv")
            nc.scalar.dma_start(bv_sb[:], bv_d[None, :].bitcast(F32R))
            ones_sb = const.tile([1, P], F32R, tag="ones")
            nc.scalar.dma_start(ones_sb[:], ones_d[None, :].bitcast(F32R))

            kt_sb = const.tile([P, 2, N], F32R, tag="kt")          # K^T, d' on part
            v_sb = const.tile([P, NKT, HG, DK + 1], F32R, tag="v")  # [V_h | 1]
            nc.scalar.dma_start(
                v_sb[:, :, :, DK].rearrange("p a b -> p (a b)"),
                ones_d[:, None].to_broadcast((P, NKT * HG)).bitcast(F32R),
            )

            def qt_proj(c, qt):
                for qh in range(2):
                    xq = xin.tile([P, KT, 512], F32R, tag="xchunk",
                                  name=f"xq_{c}_{qh}")
                    nc.sync.dma_start(xq[:], xqT[c * 2 + qh].bitcast(F32R))
                    for j in range(2):
                        ps = big.tile([P, 512], F32, tag="big",
                                      name=f"qps_{c}_{qh}_{j}")
                        for k in range(KT):
                            nc.tensor.matmul(
                                ps[:], wq_sb[:, k, j * P:(j + 1) * P], xq[:, k, :],
                                start=(k == 0), stop=(k == KT - 1))
                        nc.vector.tensor_tensor(
                            qt[:, j, qh * 512:(qh + 1) * 512], ps[:],
                            bq_sb[:, j, None].to_broadcast((P, 512)), ADD)

            def kt_proj(c4):
                xk = xin.tile([P, KT, 512], F32R, tag="xchunk", name=f"xk_{c4}")
                nc.sync.dma_start(xk[:], xkT[c4].bitcast(F32R))
                for j in range(2):
                    ps = big.tile([P, 512], F32, tag="big", name=f"kps_{c4}_{j}")
                    for k in range(KT):
                        nc.tensor.matmul(
                            ps[:], wk_sb[:, k, j * P:(j + 1) * P], xk[:, k, :],
                            start=(k == 0), stop=(k == KT - 1))
                    nc.vector.tensor_tensor(
                        kt_sb[:, j, c4 * 512:(c4 + 1) * 512], ps[:],
                        bk_sb[:, j, None].to_broadcast((P, 512)), ADD)

            def v_proj(nt):
                xv = xvp.tile([P, KT, P], F32R, tag="xv", name=f"xv_{nt}")
                nc.sync.dma_start(xv[:], xvT[nt].bitcast(F32R))
                ps = avp.tile([P, DG], F32, tag="av", name=f"vps_{nt}")
                for k in range(KT):
                    nc.tensor.matmul(ps[:], xv[:, k, :], wv_sb[:, k, :],
                                     start=(k == 0), stop=False)
                nc.tensor.matmul(ps[:], ones_sb[:], bv_sb[:], start=False, stop=True)
                nc.vector.tensor_copy(
                    v_sb[:, nt, :, 0:DK],
                    ps[:].rearrange("p (h d) -> p h d", h=HG))

            # ---- phase A: kT then V projections ----
            for c4 in range(4):
                kt_proj(c4)
            for nt in range(NKT):
                v_proj(nt)

            qts = {}
            for c in range(NCH):
                qts[c] = work.tile([P, 2, QC], F32R, tag="qt", name=f"qt_{c}")
            qt_proj(0, qts[0])

            for c in range(NCH):
                qt = qts[c]
                o_sb = work.tile([P, 2, QC], F32R, tag="o", name=f"o_{c}")

                for pair in range(2):  # heads (2*pair, 2*pair+1); j = pair
                    avs = [avp.tile([DK + 1, 512], F32, tag="av", name=f"av_{c}_{pair}_{i}")
                           for i in range(4)]
                    for kt in range(NKT):
                        for hp in range(2):
                            h = 2 * pair + hp
                            p0 = 64 * hp
                            st = big.tile([P, QC], F32, tag="big")
                            for qh in range(2):
                                nc.tensor.matmul(
                                    st[:, qh * 512:(qh + 1) * 512],
                                    kt_sb[p0:p0 + 64, pair, kt * P:(kt + 1) * P],
                                    qt[p0:p0 + 64, pair, qh * 512:(qh + 1) * 512],
                                    start=True, stop=True)
                            at = atp.tile([P, QC], F32R, tag="at")
                            nc.scalar.activation(at[:], st[:], EXP, scale=0.125)
                            for qh in range(2):
                                nc.tensor.matmul(
                                    avs[2 * hp + qh], v_sb[:, kt, h, :],
                                    at[:, qh * 512:(qh + 1) * 512],
                                    start=(kt == 0), stop=(kt == NKT - 1))

                    # overlap the z roundtrip of the last pair with the
                    # next chunk's qT projection on the PE stream
                    if pair == 1 and c + 1 < NCH:
                        qt_proj(c + 1, qts[c + 1])

                    # softmax denominators for this pair: 4 rows of 512.
                    # Broadcast Z via DRAM, then reciprocal_approx_fast on the
                    # broadcast tile (64 lanes) -- short latency chain.
                    r0 = (c * 2 + pair) * 4
                    z_dram = z_st[r0:r0 + 4, :]
                    for i in range(4):
                        zr = smal.tile([1, 512], F32, tag="zrow")
                        nc.vector.tensor_copy(zr[:], avs[i][DK:DK + 1, :])
                        nc.scalar.dma_start(z_dram[i:i + 1, :], zr[:])

                    for hp in range(2):
                        for qh in range(2):
                            zb = smal.tile([64, 512], F32, tag="zb")
                            nc.scalar.dma_start(
                                zb[:],
                                z_dram[2 * hp + qh, None, :].to_broadcast((64, 512)))
                            rzb = smal.tile([64, 512], F32, tag="rzb")
                            nc.vector.reciprocal_approx_fast(rzb[:], zb[:])
                            p0 = 64 * hp
                            nc.vector.tensor_tensor(
                                o_sb[p0:p0 + 64, pair, qh * 512:(qh + 1) * 512],
                                avs[2 * hp + qh][0:DK, :], rzb[:], MULT)

                # out-projection for this chunk: Y^T = Wo_g @ O_cat^T
                for m in range(8):
                    y_sb = smal.tile([P, QC], F32, tag="y")
                    for qh in range(2):
                        yps = big.tile([P, 512], F32, tag="big")
                        for j in range(2):
                            nc.tensor.matmul(
                                yps[:], wo_sb[:, j, m * P:(m + 1) * P],
                                o_sb[:, j, qh * 512:(qh + 1) * 512],
                                start=(j == 0), stop=(j == 1))
                        nc.vector.tensor_copy(y_sb[:, qh * 512:(qh + 1) * 512], yps[:])
                    nc.sync.dma_start(
                        yT_d[m * P:(m + 1) * P, c * QC:(c + 1) * QC], y_sb[:])

    nc.compile()
    return nc


def get_program():
    global _PROGRAM
    if _PROGRAM is None:
        _PROGRAM = _build_program()
    return _PROGRAM


def _tile_xT(x, nchunk, width):
    # x [n, 1024] -> x^T tiled [nchunk, 128 p, 8 k, width]
    xt = np.ascontiguousarray(x.T)                      # [1024, n]
    return np.ascontiguousarray(
        xt.reshape(KT, P, nchunk, width).transpose(2, 1, 0, 3))


def _tile_w(w_rows):
    # w_rows [256, 1024] (= W[g-slice]) -> W^T tiled [128 p, 8 k, 256]
    return np.ascontiguousarray(w_rows.T.reshape(KT, P, DG).transpose(1, 0, 2))


def make_in_maps(queries, keys, values, Wq, bq, Wk, bk, Wv, bv, Wo, bo):
    """Build per-core input dicts. Core c handles batch c//4, head group c%4."""
    f32 = np.float32
    xT = {}
    for ib in range(B):
        xT[ib] = (
            _tile_xT(np.asarray(queries[ib], f32), 4, 512),
            _tile_xT(np.asarray(keys[ib], f32), 4, 512),
            _tile_xT(np.asarray(values[ib], f32), NKT, P),
        )
    ones = np.ones((P,), f32)
    in_maps = []
    for core in range(8):
        ib, g = core // G, core % G
        sl = slice(g * DG, (g + 1) * DG)
        in_maps.append({
            "xqT": xT[ib][0], "xkT": xT[ib][1], "xvT": xT[ib][2],
            "wqT": _tile_w(Wq[sl, :]),
            "wkT": _tile_w(Wk[sl, :]),
            "wvT": _tile_w(Wv[sl, :]),
            "woT": np.ascontiguousarray(
                Wo[:, sl].T.reshape(2, P, D_MODEL).transpose(1, 0, 2)),
            "bq_s": np.ascontiguousarray(bq[sl]),
            "bk_s": np.ascontiguousarray(bk[sl]),
            "bv_s": np.ascontiguousarray(bv[sl]),
            "ones_c": ones,
        })
    return in_maps


def gather_output(results, bo):
    out = np.zeros((B, N, D_MODEL), np.float32)
    for core in range(8):
        out[core // G] += results[core]["yT"].T
    out += bo[None, None, :].astype(np.float32)
    return out


def _run(inputs, trace=False, **spmd_kwargs):
    nc = get_program()
    in_maps = make_in_maps(**inputs)
    res = run_bass_kernel_spmd(nc, in_maps, core_ids=list(range(8)),
                               trace=trace, **spmd_kwargs)
    return gather_output(res.results, inputs["bo"]), res


def kernel(**inputs) -> np.ndarray:
    out, _ = _run(inputs, trace=False)
    return out



# revision 7
# speedup vs baseline: 1.5633x; 1.5633x over previous
"""Multi-head attention (b=2, n=2048, d_model=1024, H=16, d_k=d_v=64) on 8
Trainium2 NeuronCores.

Sharding: 8 cores = 2 (batch) x 4 (head groups of 4 heads).  Each core
computes, for its batch ib and head group g (heads 4g..4g+3):

    kT = Wk_g @ x_k^T            [256, 2048]   (d' on partitions, bf16)
    V  = x_v @ Wv_g^T            [2048, 256]   (keys on partitions, bf16)
    qT = Wq_g @ x_q^T            [256, 2048]
    per q-chunk of 512, per key-block kt of 128, per head pair:
       S^T = K Q^T  via two row-tiled (K=64) concurrent matmuls -> PSUM f32
       at  = exp(S^T/8)          one ACTIVATE per pair  [128, 1024] -> bf16
       O^T += V_h^T A^T   via two col-tiled (M=64) concurrent matmuls
       Z   += 1^T A^T     via four col-tiled (M=1) matmuls (denominators)
    normalize O^T by 1/Z (Z broadcast across partitions via DRAM roundtrip),
    out-projection Y^T = Wo_g @ O_cat^T  [1024, 2048] f32.

Host sums the 4 per-group partial Y^T per batch and adds bo.

All matmuls run in bf16 (inputs quantized on host); accumulation is fp32 in
PSUM.  Softmax skips the max-subtraction: scores*scale are ~N(0,1) so exp
never overflows.  The scalar engine (exp: 16.8M elements/core at 1 elem/
lane/cycle ~= 147us) is the critical path; matmuls, DMA and vector work are
scheduled to hide underneath it.
"""

import numpy as np
from contextlib import ExitStack

import ml_dtypes

import concourse.bass as bass
import concourse.mybir as mybir
import concourse.tile as tile
from concourse import bacc
from concourse.bass_utils import run_bass_kernel_spmd

F32 = mybir.dt.float32
BF16 = mybir.dt.bfloat16
EXP = mybir.ActivationFunctionType.Exp
ADD = mybir.AluOpType.add
MULT = mybir.AluOpType.mult

D_MODEL = 1024
H = 16
DK = 64
B = 2
N = 2048           # nq = nk
G = 4              # head groups (cores per batch)
HG = H // G        # heads per group = 4
DG = HG * DK       # 256 group dims
KT = 8             # D_MODEL / 128 contraction tiles
NKT = N // 128     # 16 key blocks in attention
QC = 512           # attention q-chunk
NCH = N // QC      # 4 chunks
P = 128

_PROGRAM = None


def _build_program():
    nc = bacc.Bacc("TRN2", target_bir_lowering=False, debug=False, num_devices=8)

    # host-pretiled inputs; every DMA partition line is contiguous
    xqT = nc.dram_tensor("xqT", [P, NCH, KT, QC], BF16, kind="ExternalInput").ap()
    xkT = nc.dram_tensor("xkT", [P, NCH, KT, QC], BF16, kind="ExternalInput").ap()
    xvT = nc.dram_tensor("xvT", [P, NKT, KT, P], BF16, kind="ExternalInput").ap()
    wqT = nc.dram_tensor("wqT", [P, KT, DG], BF16, kind="ExternalInput").ap()
    wkT = nc.dram_tensor("wkT", [P, KT, DG], BF16, kind="ExternalInput").ap()
    wvT = nc.dram_tensor("wvT", [P, KT, DG], BF16, kind="ExternalInput").ap()
    woT = nc.dram_tensor("woT", [P, 2, D_MODEL], BF16, kind="ExternalInput").ap()
    bq_d = nc.dram_tensor("bq_s", [P, 2], F32, kind="ExternalInput").ap()
    bk_d = nc.dram_tensor("bk_s", [P, 2], F32, kind="ExternalInput").ap()
    bv_d = nc.dram_tensor("bv_r", [1, DG], BF16, kind="ExternalInput").ap()
    ones_r_d = nc.dram_tensor("ones_r", [1, P], BF16, kind="ExternalInput").ap()
    ones_c_d = nc.dram_tensor("ones_c", [P, 1], BF16, kind="ExternalInput").ap()
    zeros_d = nc.dram_tensor("zeros_w", [P, P], BF16, kind="ExternalInput").ap()
    yT_d = nc.dram_tensor("yT", [D_MODEL, N], F32, kind="ExternalOutput").ap()
    # dram staging for softmax denominators (internal DRAM tiles fail to load
    # under the axon PJRT path, so an ExternalOutput buffer instead)
    z_st = nc.dram_tensor("z_st", [4 * NCH, QC], F32, kind="ExternalOutput").ap()

    with tile.TileContext(nc) as tc:
        with ExitStack() as ctx:
            const = ctx.enter_context(tc.tile_pool(name="const", bufs=1))
            xin = ctx.enter_context(tc.tile_pool(name="xin", bufs=1))
            pers = ctx.enter_context(tc.tile_pool(name="pers", bufs=1))
            atp = ctx.enter_context(tc.tile_pool(name="atp", bufs=4))
            osb = ctx.enter_context(tc.tile_pool(name="osb", bufs=2))
            ysb = ctx.enter_context(tc.tile_pool(name="ysb", bufs=2))
            zsb = ctx.enter_context(tc.tile_pool(name="zsb", bufs=4))
            rzp = ctx.enter_context(tc.tile_pool(name="rzp", bufs=4))
            # PSUM: spool 2x[128,1024]f32 = 4 banks, av 2x1, z 1, y 1 = 8
            spool = ctx.enter_context(tc.tile_pool(name="spool", bufs=2, space="PSUM"))
            avp = ctx.enter_context(tc.tile_pool(name="avp", bufs=2, space="PSUM"))
            zp = ctx.enter_context(tc.tile_pool(name="zp", bufs=1, space="PSUM"))
            yp = ctx.enter_context(tc.tile_pool(name="yp", bufs=1, space="PSUM"))

            # ---- constants (scalar HWDGE queue; x loads go on sync) ----
            wk_sb = const.tile([P, KT, DG], BF16, tag="wk")
            wv_sb = const.tile([P, KT, DG], BF16, tag="wv")
            wq_sb = const.tile([P, KT, DG], BF16, tag="wq")
            wo_sb = const.tile([P, 2, D_MODEL], BF16, tag="wo")
            nc.scalar.dma_start(wk_sb[:], wkT)
            nc.scalar.dma_start(wv_sb[:], wvT)
            nc.scalar.dma_start(wq_sb[:], wqT)
            nc.scalar.dma_start(wo_sb[:], woT)
            bq_sb = const.tile([P, 2], F32, tag="bq")
            bk_sb = const.tile([P, 2], F32, tag="bk")
            bv_sb = const.tile([1, DG], BF16, tag="bv")
            ones_r = const.tile([1, P], BF16, tag="onr")
            ones_c = const.tile([P, 1], BF16, tag="onc")
            zeros_w = const.tile([P, P], BF16, tag="zw")
            nc.scalar.dma_start(bq_sb[:], bq_d)
            nc.scalar.dma_start(bk_sb[:], bk_d)
            nc.scalar.dma_start(bv_sb[:], bv_d)
            nc.scalar.dma_start(ones_r[:], ones_r_d)
            nc.scalar.dma_start(ones_c[:], ones_c_d)
            nc.scalar.dma_start(zeros_w[:], zeros_d)

            # ---- bulk x loads (sync HWDGE queue), 1 MiB each ----
            xk_sb = xin.tile([P, NCH, KT, QC], BF16, tag="xk")
            xv_sb = xin.tile([P, NKT, KT, P], BF16, tag="xv")
            xq_sb = xin.tile([P, NCH, KT, QC], BF16, tag="xq")
            for c in range(NCH):
                nc.sync.dma_start(xk_sb[:, c], xkT[:, c])
            for i in range(4):
                nc.sync.dma_start(xv_sb[:, 4 * i:4 * (i + 1)], xvT[:, 4 * i:4 * (i + 1)])
            for c in range(NCH):
                nc.sync.dma_start(xq_sb[:, c], xqT[:, c])

            # ---- persistent activations ----
            kt_sb = pers.tile([P, 2, N], BF16, tag="kt")     # K^T, d' on part
            v_sb = pers.tile([P, NKT, HG, DK], BF16, tag="v")  # V, keys on part
            qt_sb = pers.tile([P, 2, N], BF16, tag="qt")     # Q^T

            def k_proj(c, pools):
                for j in range(2):
                    pool, tg = pools[j % len(pools)]
                    ps = pool.tile([P, QC], F32, tag=tg, name=f"kps_{c}_{j}")
                    for k in range(KT):
                        nc.tensor.matmul(
                            ps[:], wk_sb[:, k, j * P:(j + 1) * P], xk_sb[:, c, k, :],
                            start=(k == 0), stop=(k == KT - 1))
                    nc.vector.tensor_tensor(
                        kt_sb[:, j, c * QC:(c + 1) * QC], ps[:],
                        bk_sb[:, j, None].to_broadcast((P, QC)), ADD)

            def q_proj(c, pools, half=None):
                for j in ((0, 1) if half is None else (half,)):
                    pool, tg = pools[j % len(pools)]
                    ps = pool.tile([P, QC], F32, tag=tg, name=f"qps_{c}_{j}")
                    for k in range(KT):
                        nc.tensor.matmul(
                            ps[:], wq_sb[:, k, j * P:(j + 1) * P], xq_sb[:, c, k, :],
                            start=(k == 0), stop=(k == KT - 1))
                    nc.vector.tensor_tensor(
                        qt_sb[:, j, c * QC:(c + 1) * QC], ps[:],
                        bq_sb[:, j, None].to_broadcast((P, QC)), ADD)

            def v_proj(nt, pools):
                pool, tg = pools[nt % len(pools)]
                ps = pool.tile([P, QC], F32, tag=tg, name=f"vps_{nt}")
                for k in range(KT):
                    nc.tensor.matmul(ps[:, 0:DG], xv_sb[:, nt, k, :], wv_sb[:, k, :],
                                     start=(k == 0), stop=False)
                nc.tensor.matmul(ps[:, 0:DG], ones_r[:], bv_sb[:],
                                 start=False, stop=True)
                nc.vector.tensor_copy(
                    v_sb[:, nt], ps[:, 0:DG].rearrange("p (h d) -> p h d", h=HG))

            def y_tile(c, m, pools):
                # out-projection m-tile of chunk c: Y^T[m*128:+128, cQC:+QC]
                pool, tg = pools[m % len(pools)]
                yps = pool.tile([P, QC], F32, tag=tg, name=f"yps_{c}_{m}")
                o_c = o_tiles[c]
                for j in range(2):
                    nc.tensor.matmul(
                        yps[:], wo_sb[:, j, m * P:(m + 1) * P], o_c[:, j, :],
                        start=(j == 0), stop=(j == 1))
                y_sb = ysb.tile([P, QC], F32, tag="ysb", name=f"ysb_{c}_{m}")
                nc.vector.tensor_copy(y_sb[:], yps[:])
                nc.gpsimd.dma_start(
                    yT_d[m * P:(m + 1) * P, c * QC:(c + 1) * QC], y_sb[:])

            YZ = [(yp, "y"), (zp, "z")]   # both 1-bank pools (warmup/tail only)
            YO = [(yp, "y")]              # in-chunk work must not touch zp

            # ---- warmup: K chunk 0, V block 0, Q chunk 0 ----
            k_proj(0, YZ)
            v_proj(0, YZ)
            q_proj(0, YZ)

            o_tiles = {}
            avs = {}

            def s_exp(c, kt):
                # S^T for one key block: 2 pairs x 2 row-tiled matmuls + exp
                ats = []
                for pair in range(2):
                    sps = spool.tile([P, 2 * QC], F32, tag="s",
                                     name=f"sps_{c}_{kt}_{pair}")
                    for hp in range(2):
                        p0 = 64 * hp
                        nc.tensor.matmul(
                            sps[:, hp * QC:(hp + 1) * QC],
                            kt_sb[p0:p0 + 64, pair, kt * P:(kt + 1) * P],
                            qt_sb[p0:p0 + 64, pair, c * QC:(c + 1) * QC],
                            start=True, stop=True,
                            tile_position=(p0, 0))
                    at = atp.tile([P, 2 * QC], BF16, tag="at",
                                  name=f"at_{c}_{kt}_{pair}")
                    nc.scalar.activation(at[:], sps[:], EXP, scale=0.125)
                    ats.append(at)
                return ats

            for c in range(NCH):
                av0 = avp.tile([P, QC], F32, tag="av", name=f"av0_{c}")
                av1 = avp.tile([P, QC], F32, tag="av", name=f"av1_{c}")
                zps = zp.tile([P, QC], F32, tag="z", name=f"zps_{c}")
                avs[c] = (av0, av1)

                # S/exp runs one key-block ahead of AV so the accumulator-
                # reuse wait (normalize of chunk c-1) never starves the ACT.
                ats_next = s_exp(c, 0)

                for kt in range(NKT):
                    ats = ats_next

                    if kt == 0:
                        # zero-matmuls set has_written across each whole bank
                        # so the col-tiled groups below can accumulate
                        rhs0 = xk_sb[:, 0, 0, :]
                        nc.tensor.matmul(av0[:], zeros_w[:], rhs0,
                                         start=True, stop=False)
                        nc.tensor.matmul(av1[:], zeros_w[:], rhs0,
                                         start=True, stop=False)
                        nc.tensor.matmul(zps[:], zeros_w[:], rhs0,
                                         start=True, stop=False)

                    # -- interleaved projection / output work on the PE --
                    if c == 0:
                        if kt == 1:
                            k_proj(1, YO)
                        if kt == 5:
                            k_proj(2, YO)
                        if kt == 9:
                            k_proj(3, YO)
                        if kt < NKT - 1:
                            v_proj(kt + 1, YO)
                    if c > 0 and 3 <= kt <= 10:
                        y_tile(c - 1, kt - 3, YO)
                    if c < NCH - 1 and kt in (12, 13):
                        q_proj(c + 1, YO, half=kt - 12)

                    if kt + 1 < NKT:
                        ats_next = s_exp(c, kt + 1)

                    last = kt == NKT - 1
                    for pair in range(2):
                        at = ats[pair]
                        av = avs[c][pair]
                        for hp in range(2):
                            h = 2 * pair + hp
                            nc.tensor.matmul(
                                av[64 * hp:64 * hp + 64, :],
                                v_sb[:, kt, h, :], at[:, hp * QC:(hp + 1) * QC],
                                start=False, stop=(last and hp == 1),
                                tile_position=(0, 64 * hp))
                        for hp in range(2):
                            h = 2 * pair + hp
                            nc.tensor.matmul(
                                zps[32 * h:32 * h + 1, :],
                                ones_c[:], at[:, hp * QC:(hp + 1) * QC],
                                start=False, stop=(last and pair == 1 and hp == 1),
                                tile_position=(0, 32 * h))

                # -- softmax denominators: stage via DRAM to broadcast --
                z_dram = z_st[4 * c:4 * c + 4, :]
                for h in range(4):
                    zr = zsb.tile([1, QC], F32, tag="zr", name=f"zr_{c}_{h}")
                    nc.vector.tensor_copy(zr[:], zps[32 * h:32 * h + 1, :])
                    nc.sync.dma_start(z_dram[h:h + 1, :], zr[:])

                o_sb = osb.tile([P, 2, QC], BF16, tag="o", name=f"o_{c}")
                o_tiles[c] = o_sb
                for pair in range(2):
                    for hp in range(2):
                        h = 2 * pair + hp
                        zb = rzp.tile([64, QC], F32, tag="zb", name=f"zb_{c}_{h}")
                        nc.sync.dma_start(
                            zb[:], z_dram[h, None, :].to_broadcast((64, QC)))
                        rzb = rzp.tile([64, QC], F32, tag="rzb", name=f"rzb_{c}_{h}")
                        nc.vector.reciprocal_approx_fast(rzb[:], zb[:])
                        p0 = 64 * hp
                        nc.vector.tensor_tensor(
                            o_sb[p0:p0 + 64, pair, :],
                            avs[c][pair][p0:p0 + 64, :], rzb[:], MULT)

            # tail: out-projection for the last chunk
            for m in range(8):
                y_tile(NCH - 1, m, YZ)

    nc.compile()
    return nc


def get_program():
    global _PROGRAM
    if _PROGRAM is None:
        _PROGRAM = _build_program()
    return _PROGRAM


BF = ml_dtypes.bfloat16


def _tile_xT(x, nchunk, width):
    # x [n, 1024] -> x^T tiled [128 p, nchunk, 8 k, width] bf16
    xt = np.ascontiguousarray(x.T)                      # [1024, n]
    return np.ascontiguousarray(
        xt.reshape(KT, P, nchunk, width).transpose(1, 2, 0, 3).astype(BF))


def _tile_w(w_rows):
    # w_rows [256, 1024] (= W[g-slice]) -> W^T tiled [128 p, 8 k, 256] bf16
    return np.ascontiguousarray(
        w_rows.T.reshape(KT, P, DG).transpose(1, 0, 2).astype(BF))


def make_in_maps(queries, keys, values, Wq, bq, Wk, bk, Wv, bv, Wo, bo):
    """Build per-core input dicts. Core c handles batch c//4, head group c%4."""
    f32 = np.float32
    xT = {}
    for ib in range(B):
        xT[ib] = (
            _tile_xT(np.asarray(queries[ib], f32), NCH, QC),
            _tile_xT(np.asarray(keys[ib], f32), NCH, QC),
            _tile_xT(np.asarray(values[ib], f32), NKT, P),
        )
    ones_r = np.ones((1, P), BF)
    ones_c = np.ones((P, 1), BF)
    zeros_w = np.zeros((P, P), BF)
    in_maps = []
    for core in range(8):
        ib, g = core // G, core % G
        sl = slice(g * DG, (g + 1) * DG)
        in_maps.append({
            "xqT": xT[ib][0], "xkT": xT[ib][1], "xvT": xT[ib][2],
            "wqT": _tile_w(Wq[sl, :]),
            "wkT": _tile_w(Wk[sl, :]),
            "wvT": _tile_w(Wv[sl, :]),
            "woT": np.ascontiguousarray(
                Wo[:, sl].T.reshape(2, P, D_MODEL).transpose(1, 0, 2).astype(BF)),
            "bq_s": np.ascontiguousarray(np.asarray(bq[sl], f32).reshape(2, P).T),
            "bk_s": np.ascontiguousarray(np.asarray(bk[sl], f32).reshape(2, P).T),
            "bv_r": np.ascontiguousarray(np.asarray(bv[sl], f32)[None, :].astype(BF)),
            "ones_r": ones_r,
            "ones_c": ones_c,
            "zeros_w": zeros_w,
        })
    return in_maps


def gather_output(results, bo):
    out = np.zeros((B, N, D_MODEL), np.float32)
    for core in range(8):
        out[core // G] += np.asarray(results[core]["yT"], np.float32).T
    out += np.asarray(bo, np.float32)[None, None, :]
    return out


def _run(inputs, trace=False, **spmd_kwargs):
    nc = get_program()
    in_maps = make_in_maps(**inputs)
    res = run_bass_kernel_spmd(nc, in_maps, core_ids=list(range(8)),
                               trace=trace, **spmd_kwargs)
    return gather_output(res.results, inputs["bo"]), res


def kernel(**inputs) -> np.ndarray:
    out, _ = _run(inputs, trace=False)
    return out


# revision 15
# speedup vs baseline: 1.8567x; 1.1877x over previous
"""Multi-head attention (b=2, n=2048, d_model=1024, H=16, d_k=d_v=64) on 8
Trainium2 NeuronCores.

Sharding: 8 cores = 2 (batch) x 4 (head groups of 4 heads).  Each core
computes, for its batch ib and head group g (heads 4g..4g+3):

    kT = Wk_g @ x_k^T            [256, 2048]   (d' on partitions, bf16)
    V  = x_v @ Wv_g^T            [2048, 256]   (keys on partitions, bf16)
    qT = Wq_g @ x_q^T            [256, 2048]
    per q-chunk of 512, per key-block kt of 128, per head pair:
       S^T = K Q^T  via two row-tiled (K=64) concurrent matmuls -> PSUM f32
       at  = exp(S^T/8)          one ACTIVATE per pair  [128, 1024] -> bf16
       O^T += V_h^T A^T   via two col-tiled (M=64) concurrent matmuls
       Z   += 1^T A^T     via four col-tiled (M=1) matmuls (denominators)
    1/Z broadcast across partitions via a DRAM roundtrip (DRE replicate),
    normalize O^T, out-projection Y^T = Wo_g @ O_cat^T  [1024, 2048] f32.

Host sums the 4 per-group partial Y^T per batch and adds bo.

All matmuls run in bf16 (inputs quantized on host); accumulation is fp32 in
PSUM.  Softmax skips the max-subtraction: scores*scale are ~N(0,1) so exp
never overflows.  The scalar engine (exp: 16.8M elements/core at 1 elem/
lane/cycle ~= 145us) is the critical path; matmuls, DMA and vector work are
scheduled to hide underneath it.  DMAs are issued in first-use order so the
first exp fires ~15us in.
"""

import numpy as np
from contextlib import ExitStack

import ml_dtypes

import concourse.bass as bass
import concourse.mybir as mybir
import concourse.tile as tile
from concourse import bacc
from concourse.bass_utils import run_bass_kernel_spmd

F32 = mybir.dt.float32
BF16 = mybir.dt.bfloat16
EXP = mybir.ActivationFunctionType.Exp
ADD = mybir.AluOpType.add
MULT = mybir.AluOpType.mult

D_MODEL = 1024
H = 16
DK = 64
B = 2
N = 2048           # nq = nk
G = 4              # head groups (cores per batch)
HG = H // G        # heads per group = 4
DG = HG * DK       # 256 group dims
KT = 8             # D_MODEL / 128 contraction tiles
NKT = N // 128     # 16 key blocks in attention
QC = 512           # attention q-chunk
NCH = N // QC      # 4 chunks
P = 128

_PROGRAM = None


def _build_program():
    nc = bacc.Bacc("TRN2", target_bir_lowering=False, debug=False, num_devices=8)

    # host-pretiled inputs; every DMA partition line is contiguous
    xqT = nc.dram_tensor("xqT", [P, NCH, KT, QC], BF16, kind="ExternalInput").ap()
    xkT = nc.dram_tensor("xkT", [P, NCH, KT, QC], BF16, kind="ExternalInput").ap()
    xvT = nc.dram_tensor("xvT", [P, NKT, KT, P], BF16, kind="ExternalInput").ap()
    wqT = nc.dram_tensor("wqT", [P, KT, DG], BF16, kind="ExternalInput").ap()
    wkT = nc.dram_tensor("wkT", [P, KT, DG], BF16, kind="ExternalInput").ap()
    wvT = nc.dram_tensor("wvT", [P, KT, DG], BF16, kind="ExternalInput").ap()
    woT = nc.dram_tensor("woT", [P, 2, D_MODEL], BF16, kind="ExternalInput").ap()
    bq_d = nc.dram_tensor("bq_s", [P, 2], F32, kind="ExternalInput").ap()
    bk_d = nc.dram_tensor("bk_s", [P, 2], F32, kind="ExternalInput").ap()
    bv_d = nc.dram_tensor("bv_r", [1, DG], BF16, kind="ExternalInput").ap()
    ones_r_d = nc.dram_tensor("ones_r", [1, P], BF16, kind="ExternalInput").ap()
    ones_c_d = nc.dram_tensor("ones_c", [P, 1], BF16, kind="ExternalInput").ap()
    zeros_d = nc.dram_tensor("zeros_w", [P, P], BF16, kind="ExternalInput").ap()
    yT_d = nc.dram_tensor("yT", [D_MODEL, N], F32, kind="ExternalOutput").ap()
    # dram staging for softmax denominators (internal DRAM tiles fail to load
    # under the axon PJRT path, so an ExternalOutput buffer instead)
    z_st = nc.dram_tensor("z_st", [4 * NCH, QC], F32, kind="ExternalOutput").ap()

    with tile.TileContext(nc) as tc:
        with ExitStack() as ctx:
            const = ctx.enter_context(tc.tile_pool(name="const", bufs=1))
            xin = ctx.enter_context(tc.tile_pool(name="xin", bufs=1))
            pers = ctx.enter_context(tc.tile_pool(name="pers", bufs=1))
            atp = ctx.enter_context(tc.tile_pool(name="atp", bufs=4))
            osb = ctx.enter_context(tc.tile_pool(name="osb", bufs=2))
            ysb = ctx.enter_context(tc.tile_pool(name="ysb", bufs=3))
            rzs = ctx.enter_context(tc.tile_pool(name="rzs", bufs=4))
            # PSUM: spool 2x[128,1024]f32 = 4 banks, av 2x1, z 1, y 1 = 8
            spool = ctx.enter_context(tc.tile_pool(name="spool", bufs=2, space="PSUM"))
            avp = ctx.enter_context(tc.tile_pool(name="avp", bufs=2, space="PSUM"))
            zp = ctx.enter_context(tc.tile_pool(name="zp", bufs=1, space="PSUM"))
            yp = ctx.enter_context(tc.tile_pool(name="yp", bufs=1, space="PSUM"))

            # ---- constants (scalar HWDGE queue; x loads go on sync) ----
            bq_sb = const.tile([P, 2], F32, tag="bq")
            nc.scalar.dma_start(bq_sb[:], bq_d)

            wk_sb = const.tile([P, KT, DG], BF16, tag="wk")
            wq_sb = const.tile([P, KT, DG], BF16, tag="wq")
            wv_sb = const.tile([P, KT, DG], BF16, tag="wv")
            wo_sb = const.tile([P, 2, D_MODEL], BF16, tag="wo")
            nc.scalar.dma_start(wk_sb[:], wkT)
            # touch exp so its ACT table set loads during warmup
            dum = const.tile([1, 2], F32, tag="dum")
            nc.scalar.activation(dum[:], bq_sb[0:1, :], EXP, scale=0.0)
            nc.scalar.dma_start(wq_sb[:], wqT)
            nc.scalar.dma_start(wv_sb[:], wvT)
            nc.scalar.dma_start(wo_sb[:], woT)
            bk_sb = const.tile([P, 2], F32, tag="bk")
            bv_sb = const.tile([1, DG], BF16, tag="bv")
            ones_r = const.tile([1, P], BF16, tag="onr")
            ones_c = const.tile([P, 1], BF16, tag="onc")
            zeros_w = const.tile([P, P], BF16, tag="zw")
            nc.scalar.dma_start(bk_sb[:], bk_d)
            nc.scalar.dma_start(bv_sb[:], bv_d)
            nc.scalar.dma_start(ones_r[:], ones_r_d)
            nc.scalar.dma_start(ones_c[:], ones_c_d)
            nc.scalar.dma_start(zeros_w[:], zeros_d)

            # ---- x loads (sync HWDGE queue) in first-use order ----
            xk_sb = xin.tile([P, NCH, KT, QC], BF16, tag="xk")
            xv_sb = xin.tile([P, NKT, KT, P], BF16, tag="xv")
            xq_sb = xin.tile([P, NCH, KT, QC], BF16, tag="xq")
            nc.sync.dma_start(xk_sb[:, 0], xkT[:, 0])
            nc.sync.dma_start(xq_sb[:, 0], xqT[:, 0])
            nc.sync.dma_start(xv_sb[:, 0:4], xvT[:, 0:4])
            nc.sync.dma_start(xk_sb[:, 1], xkT[:, 1])
            nc.sync.dma_start(xv_sb[:, 4:8], xvT[:, 4:8])
            nc.sync.dma_start(xk_sb[:, 2], xkT[:, 2])
            nc.sync.dma_start(xv_sb[:, 8:12], xvT[:, 8:12])
            nc.sync.dma_start(xk_sb[:, 3], xkT[:, 3])
            nc.sync.dma_start(xv_sb[:, 12:16], xvT[:, 12:16])
            nc.sync.dma_start(xq_sb[:, 1], xqT[:, 1])
            nc.sync.dma_start(xq_sb[:, 2], xqT[:, 2])
            nc.sync.dma_start(xq_sb[:, 3], xqT[:, 3])

            # ---- persistent activations ----
            kt_sb = pers.tile([P, 2, N], BF16, tag="kt")     # K^T, d' on part
            v_sb = pers.tile([P, NKT, HG, DK], BF16, tag="v")  # V, keys on part
            qt_sb = pers.tile([P, 2, N], BF16, tag="qt")     # Q^T

            YZ = [(yp, "y"), (zp, "z")]   # both 1-bank pools (warmup/tail only)
            YO = [(yp, "y")]              # in-chunk work must not touch zp

            def k_proj(c, pools, half=None):
                for j in ((0, 1) if half is None else (half,)):
                    pool, tg = pools[j % len(pools)]
                    ps = pool.tile([P, QC], F32, tag=tg, name=f"kps_{c}_{j}")
                    for k in range(KT):
                        nc.tensor.matmul(
                            ps[:], wk_sb[:, k, j * P:(j + 1) * P], xk_sb[:, c, k, :],
                            start=(k == 0), stop=(k == KT - 1))
                    nc.vector.tensor_tensor(
                        kt_sb[:, j, c * QC:(c + 1) * QC], ps[:],
                        bk_sb[:, j, None].to_broadcast((P, QC)), ADD)

            def q_proj(c, pools, half=None):
                for j in ((0, 1) if half is None else (half,)):
                    pool, tg = pools[j % len(pools)]
                    ps = pool.tile([P, QC], F32, tag=tg, name=f"qps_{c}_{j}")
                    for k in range(KT):
                        nc.tensor.matmul(
                            ps[:], wq_sb[:, k, j * P:(j + 1) * P], xq_sb[:, c, k, :],
                            start=(k == 0), stop=(k == KT - 1))
                    nc.vector.tensor_tensor(
                        qt_sb[:, j, c * QC:(c + 1) * QC], ps[:],
                        bq_sb[:, j, None].to_broadcast((P, QC)), ADD)

            def v_proj(nt, pools):
                pool, tg = pools[nt % len(pools)]
                ps = pool.tile([P, QC], F32, tag=tg, name=f"vps_{nt}")
                for k in range(KT):
                    nc.tensor.matmul(ps[:, 0:DG], xv_sb[:, nt, k, :], wv_sb[:, k, :],
                                     start=(k == 0), stop=False)
                nc.tensor.matmul(ps[:, 0:DG], ones_r[:], bv_sb[:],
                                 start=False, stop=True)
                nc.vector.tensor_copy(
                    v_sb[:, nt], ps[:, 0:DG].rearrange("p (h d) -> p h d", h=HG))

            def y_tile(c, m, pools):
                # out-projection m-tile of chunk c: Y^T[m*128:+128, cQC:+QC]
                pool, tg = pools[m % len(pools)]
                yps = pool.tile([P, QC], F32, tag=tg, name=f"yps_{c}_{m}")
                o_c = o_tiles[c]
                for j in range(2):
                    nc.tensor.matmul(
                        yps[:], wo_sb[:, j, m * P:(m + 1) * P], o_c[:, j, :],
                        start=(j == 0), stop=(j == 1))
                y_sb = ysb.tile([P, QC], F32, tag="ysb", name=f"ysb_{c}_{m}")
                nc.vector.tensor_copy(y_sb[:], yps[:])
                nc.sync.dma_start(
                    yT_d[m * P:(m + 1) * P, c * QC:(c + 1) * QC], y_sb[:])

            # ---- warmup: K chunk 0, V block 0, Q chunk 0 ----
            k_proj(0, YZ)
            q_proj(0, YZ)
            v_proj(0, YZ)

            o_tiles = {}
            avs = {}

            def s_exp(c, kt):
                # S^T for one key block: 2 pairs x 2 row-tiled matmuls + exp
                ats = []
                for pair in range(2):
                    sps = spool.tile([P, 2 * QC], F32, tag="s",
                                     name=f"sps_{c}_{kt}_{pair}")
                    for hp in range(2):
                        p0 = 64 * hp
                        nc.tensor.matmul(
                            sps[:, hp * QC:(hp + 1) * QC],
                            kt_sb[p0:p0 + 64, pair, kt * P:(kt + 1) * P],
                            qt_sb[p0:p0 + 64, pair, c * QC:(c + 1) * QC],
                            start=True, stop=True,
                            tile_position=(p0, 0))
                    at = atp.tile([P, 2 * QC], BF16, tag="at",
                                  name=f"at_{c}_{kt}_{pair}")
                    nc.scalar.activation(at[:], sps[:], EXP, scale=0.125)
                    ats.append(at)
                return ats

            ats_chunk = s_exp(0, 0)

            for c in range(NCH):
                av0 = avp.tile([P, QC], F32, tag="av", name=f"av0_{c}")
                av1 = avp.tile([P, QC], F32, tag="av", name=f"av1_{c}")
                zps = zp.tile([P, QC], F32, tag="z", name=f"zps_{c}")
                avs[c] = (av0, av1)

                # S/exp runs one key-block ahead of AV so accumulator-reuse
                # waits (normalize of chunk c-1) never starve the ACT.
                ats_next = ats_chunk

                for kt in range(NKT):
                    ats = ats_next

                    # -- interleaved projection / output work on the PE --
                    if c == 0:
                        if kt in (1, 2):
                            k_proj(1, YO, half=kt - 1)
                        if kt in (5, 6):
                            k_proj(2, YO, half=kt - 5)
                        if kt in (9, 10):
                            k_proj(3, YO, half=kt - 9)
                        if kt < NKT - 1:
                            v_proj(kt + 1, YO)
                    if c > 0 and 3 <= kt <= 10:
                        y_tile(c - 1, kt - 3, YO)
                    if c < NCH - 1 and kt in (12, 13):
                        q_proj(c + 1, YO, half=kt - 12)

                    if kt + 1 < NKT:
                        ats_next = s_exp(c, kt + 1)

                    if kt == 0:
                        # zero-matmuls set has_written across each whole bank
                        # so the col-tiled groups below can accumulate
                        rhs0 = xk_sb[:, 0, 0, :]
                        nc.tensor.matmul(av0[:], zeros_w[:], rhs0,
                                         start=True, stop=False)
                        nc.tensor.matmul(av1[:], zeros_w[:], rhs0,
                                         start=True, stop=False)
                        nc.tensor.matmul(zps[:], zeros_w[:], rhs0,
                                         start=True, stop=False)

                    last = kt == NKT - 1
                    for pair in range(2):
                        at = ats[pair]
                        av = avs[c][pair]
                        for hp in range(2):
                            h = 2 * pair + hp
                            nc.tensor.matmul(
                                av[64 * hp:64 * hp + 64, :],
                                v_sb[:, kt, h, :], at[:, hp * QC:(hp + 1) * QC],
                                start=False, stop=(last and hp == 1),
                                tile_position=(0, 64 * hp))
                    for h in range(4):
                        nc.tensor.matmul(
                            zps[32 * h:32 * h + 1, :],
                            ones_c[:], ats[h // 2][:, (h % 2) * QC:(h % 2 + 1) * QC],
                            start=False, stop=(last and h == 3),
                            tile_position=(0, 32 * h))

                # next chunk's first S/exp goes ahead of the normalize chain
                if c + 1 < NCH:
                    ats_chunk = s_exp(c + 1, 0)

                # -- softmax denominators: stage via DRAM to broadcast --
                z_dram = z_st[4 * c:4 * c + 4, :]
                for h in range(4):
                    zr = rzs.tile([1, QC], F32, tag="zr", name=f"zr_{c}_{h}")
                    nc.vector.tensor_copy(zr[:], zps[32 * h:32 * h + 1, :])
                    nc.sync.dma_start(z_dram[h:h + 1, :], zr[:])

                o_sb = osb.tile([P, 2, QC], BF16, tag="o", name=f"o_{c}")
                o_tiles[c] = o_sb
                for pair in range(2):
                    for hp in range(2):
                        h = 2 * pair + hp
                        zb = rzs.tile([64, QC], F32, tag="zb", name=f"zb_{c}_{h}")
                        nc.sync.dma_start(
                            zb[:], z_dram[h, None, :].to_broadcast((64, QC)))
                        rzb = rzs.tile([64, QC], F32, tag="rzb",
                                       name=f"rzb_{c}_{h}")
                        nc.vector.reciprocal_approx_fast(rzb[:], zb[:])
                        nc.vector.tensor_tensor(
                            o_sb[64 * hp:64 * hp + 64, pair, :],
                            avs[c][pair][64 * hp:64 * hp + 64, :], rzb[:], MULT)

            # tail: out-projection for the last chunk
            for m in range(8):
                y_tile(NCH - 1, m, YZ)

    nc.compile()
    return nc


def get_program():
    global _PROGRAM
    if _PROGRAM is None:
        _PROGRAM = _build_program()
    return _PROGRAM


BF = ml_dtypes.bfloat16


def _tile_xT(x, nchunk, width):
    # x [n, 1024] -> x^T tiled [128 p, nchunk, 8 k, width] bf16
    xt = np.ascontiguousarray(x.T)                      # [1024, n]
    return np.ascontiguousarray(
        xt.reshape(KT, P, nchunk, width).transpose(1, 2, 0, 3).astype(BF))


def _tile_w(w_rows):
    # w_rows [256, 1024] (= W[g-slice]) -> W^T tiled [128 p, 8 k, 256] bf16
    return np.ascontiguousarray(
        w_rows.T.reshape(KT, P, DG).transpose(1, 0, 2).astype(BF))


def make_in_maps(queries, keys, values, Wq, bq, Wk, bk, Wv, bv, Wo, bo):
    """Build per-core input dicts. Core c handles batch c//4, head group c%4."""
    f32 = np.float32
    xT = {}
    for ib in range(B):
        xT[ib] = (
            _tile_xT(np.asarray(queries[ib], f32), NCH, QC),
            _tile_xT(np.asarray(keys[ib], f32), NCH, QC),
            _tile_xT(np.asarray(values[ib], f32), NKT, P),
        )
    ones_r = np.ones((1, P), BF)
    ones_c = np.ones((P, 1), BF)
    zeros_w = np.zeros((P, P), BF)
    in_maps = []
    for core in range(8):
        ib, g = core // G, core % G
        sl = slice(g * DG, (g + 1) * DG)
        in_maps.append({
            "xqT": xT[ib][0], "xkT": xT[ib][1], "xvT": xT[ib][2],
            "wqT": _tile_w(Wq[sl, :]),
            "wkT": _tile_w(Wk[sl, :]),
            "wvT": _tile_w(Wv[sl, :]),
            "woT": np.ascontiguousarray(
                Wo[:, sl].T.reshape(2, P, D_MODEL).transpose(1, 0, 2).astype(BF)),
            "bq_s": np.ascontiguousarray(np.asarray(bq[sl], f32).reshape(2, P).T),
            "bk_s": np.ascontiguousarray(np.asarray(bk[sl], f32).reshape(2, P).T),
            "bv_r": np.ascontiguousarray(np.asarray(bv[sl], f32)[None, :].astype(BF)),
            "ones_r": ones_r,
            "ones_c": ones_c,
            "zeros_w": zeros_w,
        })
    return in_maps


def gather_output(results, bo):
    out = np.zeros((B, N, D_MODEL), np.float32)
    for core in range(8):
        out[core // G] += np.asarray(results[core]["yT"], np.float32).T
    out += np.asarray(bo, np.float32)[None, None, :]
    return out


def _run(inputs, trace=False, **spmd_kwargs):
    nc = get_program()
    in_maps = make_in_maps(**inputs)
    res = run_bass_kernel_spmd(nc, in_maps, core_ids=list(range(8)),
                               trace=trace, **spmd_kwargs)
    return gather_output(res.results, inputs["bo"]), res


def kernel(**inputs) -> np.ndarray:
    out, _ = _run(inputs, trace=False)
    return out
